# revision 2
# baseline (speedup 1.0000x reference)
"""CRLLoss (majority-masked mean CE) on 8 trn2 NeuronCores — v6 (row drop).

Identical math/pipeline to kernel_new v5, plus: rows whose label is in
min_classes contribute exactly zero to both partial sums (the reference
multiplies them by 0), so the host drops them before upload — the classic
ignore-index CE optimization. Kept rows are re-sharded evenly across the 8
cores, padded (x=0, keep=0) to a rectangular [128 x Gtot] layout, and the
kernel is built per padded-geometry (cached; ~10% fewer groups for a 10%
minority share).

Per-core pipeline (see kernel_new v5 docstring for the full rationale):
  fp8->ScalarE-exp rows + fp16->DVE-Schraudolph rows (4x tensor_scalar ops,
  int16 round-to-nearest bit trick verified on HW), per-row-group 4x-mode
  accumulates into fp32 sumexp, emission-ordered columns with a phased
  Ln + keep*lnZ tensor_tensor_reduce epilogue, ACT-accum final chunk, and a
  [128, 2] per-partition partial pair summed on host.
"""

import os
import numpy as np
import ml_dtypes

import concourse.bass as bass
import concourse.tile as tile
from concourse import bacc, mybir
from concourse.bass_utils import run_bass_kernel_spmd

LOSS_WEIGHT = 1.0

N, C = 262144, 1000
NCORES = 8
P = 128                     # SBUF partitions
A_SCH = 1477.3196           # 1024 * log2(e)
B_SCH = 15300.7             # bias tuned for zero-mean relative error
F8_SHARE = 0.594            # fp8/ACT row-group share
PHASE_LAG = 4

_F32 = mybir.dt.float32
_F16 = mybir.dt.float16
_F8 = mybir.dt.float8e4
_I16 = mybir.dt.int16

_cached = {}
_cached_nc = None          # last-built nc (harness/test introspection)


def _geom(gtot):
    """(rpp8, rpp16, a_sizes, d_sizes) for gtot row-groups per partition."""
    assert gtot % 8 == 0 and gtot >= 32
    rpp8 = int(round(gtot * F8_SHARE / 8)) * 8
    rpp8 = max(16, min(gtot - 16, rpp8))
    rpp16 = gtot - rpp8
    a_sizes = [8] * (rpp8 // 8)
    d_sizes = [4] + [8] * ((rpp16 - 8) // 8) + [4]
    return rpp8, rpp16, a_sizes, d_sizes


def _col_layout(gtot):
    """Emission-ordered columns: list of (kind, own_lo, ng, col)."""
    rpp8, rpp16, a_sizes, d_sizes = _geom(gtot)
    a, lo = [], 0
    for n in a_sizes:
        a.append(("a", lo, n)); lo += n
    d, lo = [], 0
    for n in d_sizes:
        d.append(("d", lo, n)); lo += n
    order = [d[0], a[0]]
    mid_a, mid_d = a[1:-1], d[1:-1]
    i = j = 0
    while i < len(mid_a) or j < len(mid_d):
        if (i * len(mid_d) <= j * len(mid_a) and i < len(mid_a)) or j >= len(mid_d):
            order.append(mid_a[i]); i += 1
        else:
            order.append(mid_d[j]); j += 1
    order += [d[-1], a[-1]]
    out, col = [], 0
    for kind, own_lo, ng in order:
        out.append((kind, own_lo, ng, col)); col += ng
    assert col == gtot
    return out


def _build_nc(gtot):
    rpp8, rpp16, a_sizes, d_sizes = _geom(gtot)
    nc = bacc.Bacc("TRN2", debug=False, target_bir_lowering=False)

    x8 = nc.dram_tensor("x8", [P * rpp8, C], _F8, kind="ExternalInput")
    x16 = nc.dram_tensor("x16", [P * rpp16, C], _F16, kind="ExternalInput")
    gathf = nc.dram_tensor("gathf", [P, gtot], _F32, kind="ExternalInput")
    keepf = nc.dram_tensor("keepf", [P, gtot], _F32, kind="ExternalInput")
    out = nc.dram_tensor("out", [P, 2], _F32, kind="ExternalOutput")

    x8r = x8.ap().rearrange("(p r) c -> p r c", p=P)
    x16r = x16.ap().rearrange("(p r) c -> p r c", p=P)

    layout = _col_layout(gtot)
    ncols_done_after = []
    cum = 0
    for _, _, ng, _ in layout:
        cum += ng
        ncols_done_after.append(cum)

    with tile.TileContext(nc) as tc:
        with (
            tc.tile_pool(name="x8p", bufs=4) as x8p,
            tc.tile_pool(name="e16p", bufs=3) as e16p,
            tc.tile_pool(name="x16p", bufs=3) as x16p,
            tc.tile_pool(name="bitp", bufs=2) as bitp,
            tc.tile_pool(name="consts", bufs=1) as consts,
        ):
            # one activation table with BOTH exp and ln (no mid-stream swaps)
            nc.scalar.add_instruction(mybir.InstLoadActFuncSet(
                name=nc.get_next_instruction_name(), ins=[], outs=[],
                act_func_set_id=6))

            keep_s = consts.tile([P, gtot], _F32)
            gath_s = consts.tile([P, gtot], _F32)
            sumexp = consts.tile([P, gtot], _F32)
            dummy = consts.tile([P, C], _F16, tag="dummy")
            dummy32 = consts.tile([P, gtot], _F32, tag="dummy32")
            part = consts.tile([P, 2], _F32)
            keepg_neg = consts.tile([P, 1], _F32)

            def emit_consts():
                nc.sync.dma_start(keep_s[:], keepf.ap())
                nc.sync.dma_start(gath_s[:], gathf.ap())
                nc.vector.tensor_reduce(
                    part[:, 1:2], keep_s[:], axis=mybir.AxisListType.X,
                    op=mybir.AluOpType.add)
                nc.vector.tensor_tensor(
                    dummy32[:], keep_s[:], gath_s[:], op=mybir.AluOpType.mult)
                nc.vector.tensor_scalar(
                    dummy32[:], dummy32[:], -1.0, 0.0,
                    op0=mybir.AluOpType.mult, op1=mybir.AluOpType.add,
                    accum_out=keepg_neg[:])

            def emit_a(own_lo, ng, col, act_accum=False, split=None):
                xt = x8p.tile([P, ng, C], _F8)
                nc.sync.dma_start(xt[:], x8r[:, own_lo:own_lo + ng, :])
                if act_accum:
                    for k in range(ng):
                        et = e16p.tile([P, 1, C], _F16, tag="etacc")
                        nc.scalar.activation(
                            et[:, 0, :], xt[:, k, :],
                            mybir.ActivationFunctionType.Exp,
                            accum_out=sumexp[:, col + k:col + k + 1])
                    return
                et = e16p.tile([P, ng, C], _F16)
                k0 = 0
                for sub in (split or [ng]):
                    nc.scalar.activation(
                        et[:, k0:k0 + sub, :], xt[:, k0:k0 + sub, :],
                        mybir.ActivationFunctionType.Exp)
                    for k in range(k0, k0 + sub):
                        j = col + k
                        nc.vector.tensor_scalar(
                            dummy[:], et[:, k, :], 1.0, 0.0,
                            op0=mybir.AluOpType.mult, op1=mybir.AluOpType.add,
                            accum_out=sumexp[:, j:j + 1])
                    k0 += sub

            def emit_d(own_lo, ng, col):
                xt = x16p.tile([P, ng, C], _F16)
                nc.sync.dma_start(xt[:], x16r[:, own_lo:own_lo + ng, :])
                bt = bitp.tile([P, ng, C], _I16)
                nc.vector.tensor_scalar(
                    bt[:], xt[:], A_SCH, B_SCH,
                    op0=mybir.AluOpType.mult, op1=mybir.AluOpType.add)
                btf = bt[:].bitcast(_F16)
                for k in range(ng):
                    j = col + k
                    nc.vector.tensor_scalar(
                        dummy[:], btf[:, k, :], 1.0, 0.0,
                        op0=mybir.AluOpType.mult, op1=mybir.AluOpType.add,
                        accum_out=sumexp[:, j:j + 1])

            # phased Ln + masked reduce; literal TTR seeds, folded by tiny adds
            ce_run = [keepg_neg[:]]

            def emit_phase(lo, hi):
                ncol = hi - lo
                logz = consts.tile([P, ncol], _F32, tag=f"logz{lo}")
                nc.scalar.activation(
                    logz[:], sumexp[:, lo:hi], mybir.ActivationFunctionType.Ln)
                ce_t = consts.tile([P, 1], _F32, tag=f"ce{lo}")
                nc.vector.tensor_tensor(
                    dummy32[:, 0:ncol], logz[:], keep_s[:, lo:hi],
                    op=mybir.AluOpType.mult)
                nc.vector.tensor_scalar(
                    dummy32[:, 0:ncol], dummy32[:, 0:ncol], 1.0, 0.0,
                    op0=mybir.AluOpType.mult, op1=mybir.AluOpType.add,
                    accum_out=ce_t[:])
                if hi == gtot:
                    dst = part[:, 0:1]
                else:
                    dst_t = consts.tile([P, 1], _F32, tag=f"cerun{lo}")
                    dst = dst_t[:]
                nc.vector.tensor_tensor(
                    dst, ce_run[0], ce_t[:], op=mybir.AluOpType.add)
                ce_run[0] = dst

            qt = max(8, gtot // 4 // 8 * 8)
            phase_targets = [qt, 2 * qt, 3 * qt, gtot - 16]
            phase_targets = sorted({t for t in phase_targets if 0 < t < gtot})
            phase_emit_after = {}
            for t in phase_targets:
                idx = next(i for i, cc in enumerate(ncols_done_after) if cc >= t)
                phase_emit_after[min(idx + PHASE_LAG, len(layout) - 1)] = t
            phase_lo = 0

            first_a = next(i for i, e in enumerate(layout) if e[0] == "a")
            for ei, (kind, own_lo, ng, col) in enumerate(layout):
                last = ei == len(layout) - 1
                if kind == "a":
                    split = [4, 4] if ei == first_a and ng == 8 else None
                    emit_a(own_lo, ng, col, act_accum=last, split=split)
                else:
                    emit_d(own_lo, ng, col)
                if ei == 1:
                    emit_consts()
                if ei in phase_emit_after:
                    t = phase_emit_after[ei]
                    emit_phase(phase_lo, t)
                    phase_lo = t

            emit_phase(phase_lo, gtot)
            nc.sync.dma_start(out.ap(), part[:])

    nc.compile()
    return nc


def kernel(cls_score, label, min_classes):
    cls_score = np.ascontiguousarray(np.asarray(cls_score, dtype=np.float32))
    label = np.asarray(label).astype(np.int64)
    min_classes = np.asarray(min_classes)

    keep = ~np.isin(label, min_classes)                        # [N] bool
    kept = np.nonzero(keep)[0]
    if kept.size == 0:
        return np.array(0.0, dtype=np.float32)

    per_core = -(-kept.size // NCORES)                         # ceil
    gtot = max(32, -(-per_core // (P * 8)) * 8)                # groups, %8
    cap = P * gtot

    global _cached_nc
    nc = _cached.get(gtot)
    if nc is None:
        nc = _cached[gtot] = _build_nc(gtot)
    _cached_nc = nc

    rpp8, rpp16, _, _ = _geom(gtot)
    layout = _col_layout(gtot)
    perm = np.empty(gtot, dtype=np.int64)
    for kind, own_lo, ng, col in layout:
        src = own_lo + (0 if kind == "a" else rpp8)
        perm[col:col + ng] = np.arange(src, src + ng)

    in_maps = []
    for s in range(NCORES):
        idx = kept[s * per_core:(s + 1) * per_core]
        n_c = idx.size
        xs = np.zeros((cap, C), dtype=np.float32)
        xs[:n_c] = cls_score[idx]
        ls = np.zeros(cap, dtype=np.int64)
        ls[:n_c] = label[idx]
        ks = np.zeros(cap, dtype=np.float32)
        ks[:n_c] = 1.0
        # partition-major [P, gtot]
        xs = xs.reshape(P, gtot, C)
        ls = ls.reshape(P, gtot)
        ks = ks.reshape(P, gtot)
        x8 = np.ascontiguousarray(xs[:, :rpp8]).reshape(P * rpp8, C)
        x16 = np.ascontiguousarray(xs[:, rpp8:]).reshape(P * rpp16, C)
        x8q = x8.astype(ml_dtypes.float8_e4m3)
        x16q = x16.astype(np.float16)
        # byte-identical to an on-device gather of the uploaded arrays
        gath_own = np.empty((P, gtot), dtype=np.float32)
        gath_own[:, :rpp8] = x8q[np.arange(P * rpp8),
                                 ls[:, :rpp8].reshape(-1)].astype(
            np.float32).reshape(P, rpp8)
        gath_own[:, rpp8:] = x16q[np.arange(P * rpp16),
                                  ls[:, rpp8:].reshape(-1)].astype(
            np.float32).reshape(P, rpp16)
        in_maps.append({
            "x8": x8q,
            "x16": x16q,
            "gathf": np.ascontiguousarray(gath_own[:, perm]),
            "keepf": np.ascontiguousarray(ks[:, perm]),
        })

    results = run_bass_kernel_spmd(nc, in_maps, core_ids=list(range(NCORES)))
    partials = np.stack([r["out"] for r in results.results])  # [8, P, 2]
    ce_sum = float(partials[:, :, 0].astype(np.float64).sum())
    keep_sum = float(partials[:, :, 1].astype(np.float64).sum())
    return np.array(LOSS_WEIGHT * ce_sum / max(keep_sum, 1.0), dtype=np.float32)


# revision 4
# speedup vs baseline: 1.0104x; 1.0104x over previous
"""CRLLoss (majority-masked mean CE) on 8 trn2 NeuronCores — v6 (row drop).

Identical math/pipeline to kernel_new v5, plus: rows whose label is in
min_classes contribute exactly zero to both partial sums (the reference
multiplies them by 0), so the host drops them before upload — the classic
ignore-index CE optimization. Kept rows are re-sharded evenly across the 8
cores, padded (x=0, keep=0) to a rectangular [128 x Gtot] layout, and the
kernel is built per padded-geometry (cached; ~10% fewer groups for a 10%
minority share).

Per-core pipeline (see kernel_new v5 docstring for the full rationale):
  fp8->ScalarE-exp rows + fp16->DVE-Schraudolph rows (4x tensor_scalar ops,
  int16 round-to-nearest bit trick verified on HW), per-row-group 4x-mode
  accumulates into fp32 sumexp, emission-ordered columns with a phased
  Ln + keep*lnZ multiply/accumulate epilogue, ACT-accum final chunk, and a
  [128, 2] per-partition partial pair summed on host.
"""

import os
import numpy as np
import ml_dtypes

import concourse.bass as bass
import concourse.tile as tile
from concourse import bacc, mybir
from concourse.bass_utils import run_bass_kernel_spmd

LOSS_WEIGHT = 1.0

N, C = 262144, 1000
NCORES = 8
P = 128                     # SBUF partitions
A_SCH = 1477.3196           # 1024 * log2(e)
B_SCH = 15300.7             # bias tuned for zero-mean relative error
F8_SHARE = 0.594            # fp8/ACT row-group share
PHASE_LAG = 4

_F32 = mybir.dt.float32
_F16 = mybir.dt.float16
_F8 = mybir.dt.float8e4
_I16 = mybir.dt.int16

_cached = {}
_cached_nc = None          # last-built nc (harness/test introspection)


def _geom(gtot):
    """(rpp8, rpp16, a_sizes, d_sizes) for gtot row-groups per partition."""
    assert gtot % 8 == 0 and gtot >= 32
    rpp8 = int(round(gtot * F8_SHARE / 8)) * 8
    rpp8 = max(16, min(gtot - 16, rpp8))
    rpp16 = gtot - rpp8
    a_sizes = [8] * (rpp8 // 8)
    d_sizes = [4] + [8] * ((rpp16 - 8) // 8) + [4]
    return rpp8, rpp16, a_sizes, d_sizes


def _col_layout(gtot):
    """Emission-ordered columns: list of (kind, own_lo, ng, col)."""
    rpp8, rpp16, a_sizes, d_sizes = _geom(gtot)
    a, lo = [], 0
    for n in a_sizes:
        a.append(("a", lo, n)); lo += n
    d, lo = [], 0
    for n in d_sizes:
        d.append(("d", lo, n)); lo += n
    order = [d[0], a[0]]
    mid_a, mid_d = a[1:-1], d[1:-1]
    i = j = 0
    while i < len(mid_a) or j < len(mid_d):
        if (i * len(mid_d) <= j * len(mid_a) and i < len(mid_a)) or j >= len(mid_d):
            order.append(mid_a[i]); i += 1
        else:
            order.append(mid_d[j]); j += 1
    order += [d[-1], a[-1]]
    out, col = [], 0
    for kind, own_lo, ng in order:
        out.append((kind, own_lo, ng, col)); col += ng
    assert col == gtot
    return out


def _build_nc(gtot):
    rpp8, rpp16, a_sizes, d_sizes = _geom(gtot)
    nc = bacc.Bacc("TRN2", debug=False, target_bir_lowering=False)

    x8 = nc.dram_tensor("x8", [P * rpp8, C], _F8, kind="ExternalInput")
    x16 = nc.dram_tensor("x16", [P * rpp16, C], _F16, kind="ExternalInput")
    gathf = nc.dram_tensor("gathf", [P, gtot], _F32, kind="ExternalInput")
    keepf = nc.dram_tensor("keepf", [P, gtot], _F32, kind="ExternalInput")
    out = nc.dram_tensor("out", [P, 2], _F32, kind="ExternalOutput")

    x8r = x8.ap().rearrange("(p r) c -> p r c", p=P)
    x16r = x16.ap().rearrange("(p r) c -> p r c", p=P)

    layout = _col_layout(gtot)
    ncols_done_after = []
    cum = 0
    for _, _, ng, _ in layout:
        cum += ng
        ncols_done_after.append(cum)

    with tile.TileContext(nc) as tc:
        with (
            tc.tile_pool(name="x8p", bufs=4) as x8p,
            tc.tile_pool(name="e16p", bufs=3) as e16p,
            tc.tile_pool(name="x16p", bufs=3) as x16p,
            tc.tile_pool(name="bitp", bufs=2) as bitp,
            tc.tile_pool(name="consts", bufs=1) as consts,
        ):
            # one activation table with BOTH exp and ln (no mid-stream swaps)
            nc.scalar.add_instruction(mybir.InstLoadActFuncSet(
                name=nc.get_next_instruction_name(), ins=[], outs=[],
                act_func_set_id=6))

            keep_s = consts.tile([P, gtot], _F32)
            gath_s = consts.tile([P, gtot], _F32)
            sumexp = consts.tile([P, gtot], _F32)
            dummy = consts.tile([P, C], _F16, tag="dummy")
            dummy_b = consts.tile([P, C], _F16, tag="dummy_b")
            dums = [dummy, dummy_b]
            dummy32 = consts.tile([P, gtot], _F32, tag="dummy32")
            part = consts.tile([P, 2], _F32)
            keepg_neg = consts.tile([P, 1], _F32)

            def emit_consts():
                nc.sync.dma_start(keep_s[:], keepf.ap())
                nc.sync.dma_start(gath_s[:], gathf.ap())
                nc.vector.tensor_reduce(
                    part[:, 1:2], keep_s[:], axis=mybir.AxisListType.X,
                    op=mybir.AluOpType.add)
                nc.vector.tensor_tensor(
                    dummy32[:], keep_s[:], gath_s[:], op=mybir.AluOpType.mult)
                nc.vector.tensor_scalar(
                    dummy32[:], dummy32[:], -1.0, 0.0,
                    op0=mybir.AluOpType.mult, op1=mybir.AluOpType.add,
                    accum_out=keepg_neg[:])

            def emit_a(own_lo, ng, col, act_accum=False, split=None):
                xt = x8p.tile([P, ng, C], _F8)
                nc.sync.dma_start(xt[:], x8r[:, own_lo:own_lo + ng, :])
                if act_accum:
                    for k in range(ng):
                        et = e16p.tile([P, 1, C], _F16, tag="etacc")
                        nc.scalar.activation(
                            et[:, 0, :], xt[:, k, :],
                            mybir.ActivationFunctionType.Exp,
                            accum_out=sumexp[:, col + k:col + k + 1])
                    return
                et = e16p.tile([P, ng, C], _F16)
                k0 = 0
                for sub in (split or [ng]):
                    nc.scalar.activation(
                        et[:, k0:k0 + sub, :], xt[:, k0:k0 + sub, :],
                        mybir.ActivationFunctionType.Exp)
                    for k in range(k0, k0 + sub):
                        j = col + k
                        nc.vector.tensor_scalar(
                            dums[j % 2][:], et[:, k, :], 1.0, 0.0,
                            op0=mybir.AluOpType.mult, op1=mybir.AluOpType.add,
                            accum_out=sumexp[:, j:j + 1])
                    k0 += sub

            def emit_d(own_lo, ng, col):
                xt = x16p.tile([P, ng, C], _F16)
                nc.sync.dma_start(xt[:], x16r[:, own_lo:own_lo + ng, :])
                bt = bitp.tile([P, ng, C], _I16)
                nc.vector.tensor_scalar(
                    bt[:], xt[:], A_SCH, B_SCH,
                    op0=mybir.AluOpType.mult, op1=mybir.AluOpType.add)
                btf = bt[:].bitcast(_F16)
                for k in range(ng):
                    j = col + k
                    nc.vector.tensor_scalar(
                        dums[j % 2][:], btf[:, k, :], 1.0, 0.0,
                        op0=mybir.AluOpType.mult, op1=mybir.AluOpType.add,
                        accum_out=sumexp[:, j:j + 1])

            # phased Ln + masked reduce; literal TTR seeds, folded by tiny adds
            ce_run = [keepg_neg[:]]

            def emit_phase(lo, hi):
                ncol = hi - lo
                logz = consts.tile([P, ncol], _F32, tag=f"logz{lo}")
                nc.scalar.activation(
                    logz[:], sumexp[:, lo:hi], mybir.ActivationFunctionType.Ln)
                ce_t = consts.tile([P, 1], _F32, tag=f"ce{lo}")
                nc.vector.tensor_tensor(
                    dummy32[:, 0:ncol], logz[:], keep_s[:, lo:hi],
                    op=mybir.AluOpType.mult)
                nc.vector.tensor_scalar(
                    dummy32[:, 0:ncol], dummy32[:, 0:ncol], 1.0, 0.0,
                    op0=mybir.AluOpType.mult, op1=mybir.AluOpType.add,
                    accum_out=ce_t[:])
                if hi == gtot:
                    dst = part[:, 0:1]
                else:
                    dst_t = consts.tile([P, 1], _F32, tag=f"cerun{lo}")
                    dst = dst_t[:]
                nc.vector.tensor_tensor(
                    dst, ce_run[0], ce_t[:], op=mybir.AluOpType.add)
                ce_run[0] = dst

            qt = max(8, gtot // 4 // 8 * 8)
            phase_targets = [qt, 2 * qt, 3 * qt, gtot - 16]
            phase_targets = sorted({t for t in phase_targets if 0 < t < gtot})
            phase_emit_after = {}
            for t in phase_targets:
                idx = next(i for i, cc in enumerate(ncols_done_after) if cc >= t)
                phase_emit_after[min(idx + PHASE_LAG, len(layout) - 1)] = t
            phase_lo = 0

            first_a = next(i for i, e in enumerate(layout) if e[0] == "a")
            for ei, (kind, own_lo, ng, col) in enumerate(layout):
                last = ei == len(layout) - 1
                if kind == "a":
                    split = [4, 4] if ei == first_a and ng == 8 else None
                    emit_a(own_lo, ng, col,
                           act_accum=False,
                           split=split)
                else:
                    emit_d(own_lo, ng, col)
                if ei == 1:
                    emit_consts()
                if ei in phase_emit_after:
                    t = phase_emit_after[ei]
                    emit_phase(phase_lo, t)
                    phase_lo = t

            emit_phase(phase_lo, gtot)
            nc.sync.dma_start(out.ap(), part[:])

    nc.compile()
    return nc


def kernel(cls_score, label, min_classes):
    cls_score = np.ascontiguousarray(np.asarray(cls_score, dtype=np.float32))
    label = np.asarray(label).astype(np.int64)
    min_classes = np.asarray(min_classes)

    keep = ~np.isin(label, min_classes)                        # [N] bool
    kept = np.nonzero(keep)[0]
    if kept.size == 0:
        return np.array(0.0, dtype=np.float32)

    per_core = -(-kept.size // NCORES)                         # ceil
    gtot = max(32, -(-per_core // (P * 8)) * 8)                # groups, %8
    cap = P * gtot

    global _cached_nc
    nc = _cached.get(gtot)
    if nc is None:
        nc = _cached[gtot] = _build_nc(gtot)
    _cached_nc = nc

    rpp8, rpp16, _, _ = _geom(gtot)
    layout = _col_layout(gtot)
    perm = np.empty(gtot, dtype=np.int64)
    for kind, own_lo, ng, col in layout:
        src = own_lo + (0 if kind == "a" else rpp8)
        perm[col:col + ng] = np.arange(src, src + ng)

    in_maps = []
    for s in range(NCORES):
        idx = kept[s * per_core:(s + 1) * per_core]
        n_c = idx.size
        xs = np.zeros((cap, C), dtype=np.float32)
        xs[:n_c] = cls_score[idx]
        ls = np.zeros(cap, dtype=np.int64)
        ls[:n_c] = label[idx]
        ks = np.zeros(cap, dtype=np.float32)
        ks[:n_c] = 1.0
        # partition-major [P, gtot]
        xs = xs.reshape(P, gtot, C)
        ls = ls.reshape(P, gtot)
        ks = ks.reshape(P, gtot)
        x8 = np.ascontiguousarray(xs[:, :rpp8]).reshape(P * rpp8, C)
        x16 = np.ascontiguousarray(xs[:, rpp8:]).reshape(P * rpp16, C)
        x8q = x8.astype(ml_dtypes.float8_e4m3)
        x16q = x16.astype(np.float16)
        # byte-identical to an on-device gather of the uploaded arrays
        gath_own = np.empty((P, gtot), dtype=np.float32)
        gath_own[:, :rpp8] = x8q[np.arange(P * rpp8),
                                 ls[:, :rpp8].reshape(-1)].astype(
            np.float32).reshape(P, rpp8)
        gath_own[:, rpp8:] = x16q[np.arange(P * rpp16),
                                  ls[:, rpp8:].reshape(-1)].astype(
            np.float32).reshape(P, rpp16)
        in_maps.append({
            "x8": x8q,
            "x16": x16q,
            "gathf": np.ascontiguousarray(gath_own[:, perm]),
            "keepf": np.ascontiguousarray(ks[:, perm]),
        })

    results = run_bass_kernel_spmd(nc, in_maps, core_ids=list(range(NCORES)))
    partials = np.stack([r["out"] for r in results.results])  # [8, P, 2]
    ce_sum = float(partials[:, :, 0].astype(np.float64).sum())
    keep_sum = float(partials[:, :, 1].astype(np.float64).sum())
    return np.array(LOSS_WEIGHT * ce_sum / max(keep_sum, 1.0), dtype=np.float32)


# revision 6
# speedup vs baseline: 1.0207x; 1.0102x over previous
"""CRLLoss (majority-masked mean CE) on 8 trn2 NeuronCores — v6 (row drop).

Identical math/pipeline to kernel_new v5, plus: rows whose label is in
min_classes contribute exactly zero to both partial sums (the reference
multiplies them by 0), so the host drops them before upload — the classic
ignore-index CE optimization. Kept rows are re-sharded evenly across the 8
cores, padded (x=0, keep=0) to a rectangular [128 x Gtot] layout, and the
kernel is built per padded-geometry (cached; ~10% fewer groups for a 10%
minority share).

Per-core pipeline (see kernel_new v5 docstring for the full rationale):
  fp8->ScalarE-exp rows + fp16->DVE-Schraudolph rows (4x tensor_scalar ops,
  int16 round-to-nearest bit trick verified on HW), per-row-group 4x-mode
  accumulates into fp32 sumexp, emission-ordered columns with a phased
  Ln + keep*lnZ multiply/accumulate epilogue, ping-ponged scratch tiles, and a
  [128, 2] per-partition partial pair summed on host.
"""

import os
import numpy as np
import ml_dtypes

import concourse.bass as bass
import concourse.tile as tile
from concourse import bacc, mybir
from concourse.bass_utils import run_bass_kernel_spmd

LOSS_WEIGHT = 1.0

N, C = 262144, 1000
NCORES = 8
P = 128                     # SBUF partitions
A_SCH = 1477.3196           # 1024 * log2(e)
B_SCH = 15300.7             # bias tuned for zero-mean relative error
F8_SHARE = 0.594            # fp8/ACT row-group share
PHASE_LAG = 4

_F32 = mybir.dt.float32
_F16 = mybir.dt.float16
_F8 = mybir.dt.float8e4
_I16 = mybir.dt.int16

_cached = {}
_cached_nc = None          # last-built nc (harness/test introspection)


def _geom(gtot):
    """(rpp8, rpp16, a_sizes, d_sizes) for gtot row-groups per partition."""
    assert gtot % 8 == 0 and gtot >= 32
    rpp8 = int(round(gtot * F8_SHARE / 8)) * 8
    rpp8 = max(16, min(gtot - 16, rpp8))
    rpp16 = gtot - rpp8
    a_sizes = [8] * (rpp8 // 8)
    d_sizes = [4] + [8] * ((rpp16 - 8) // 8) + [4]
    return rpp8, rpp16, a_sizes, d_sizes


def _col_layout(gtot):
    """Emission-ordered columns: list of (kind, own_lo, ng, col)."""
    rpp8, rpp16, a_sizes, d_sizes = _geom(gtot)
    a, lo = [], 0
    for n in a_sizes:
        a.append(("a", lo, n)); lo += n
    d, lo = [], 0
    for n in d_sizes:
        d.append(("d", lo, n)); lo += n
    order = [d[0], a[0]]
    mid_a, mid_d = a[1:-1], d[1:-1]
    i = j = 0
    while i < len(mid_a) or j < len(mid_d):
        if (i * len(mid_d) <= j * len(mid_a) and i < len(mid_a)) or j >= len(mid_d):
            order.append(mid_a[i]); i += 1
        else:
            order.append(mid_d[j]); j += 1
    order += [d[-1], a[-1]]
    out, col = [], 0
    for kind, own_lo, ng in order:
        out.append((kind, own_lo, ng, col)); col += ng
    assert col == gtot
    return out


def _build_nc(gtot):
    rpp8, rpp16, a_sizes, d_sizes = _geom(gtot)
    nc = bacc.Bacc("TRN2", debug=False, target_bir_lowering=False)

    x8 = nc.dram_tensor("x8", [P * rpp8, C], _F8, kind="ExternalInput")
    x16 = nc.dram_tensor("x16", [P * rpp16, C], _F16, kind="ExternalInput")
    gathf = nc.dram_tensor("gathf", [P, gtot], _F32, kind="ExternalInput")
    keepf = nc.dram_tensor("keepf", [P, gtot], _F32, kind="ExternalInput")
    out = nc.dram_tensor("out", [P, 2], _F32, kind="ExternalOutput")

    x8r = x8.ap().rearrange("(p r) c -> p r c", p=P)
    x16r = x16.ap().rearrange("(p r) c -> p r c", p=P)

    layout = _col_layout(gtot)
    ncols_done_after = []
    cum = 0
    for _, _, ng, _ in layout:
        cum += ng
        ncols_done_after.append(cum)

    with tile.TileContext(nc) as tc:
        with (
            tc.tile_pool(name="x8p", bufs=4) as x8p,
            tc.tile_pool(name="e16p", bufs=3) as e16p,
            tc.tile_pool(name="x16p", bufs=3) as x16p,
            tc.tile_pool(name="bitp", bufs=2) as bitp,
            tc.tile_pool(name="consts", bufs=1) as consts,
        ):
            # one activation table with BOTH exp and ln (no mid-stream swaps)
            nc.scalar.add_instruction(mybir.InstLoadActFuncSet(
                name=nc.get_next_instruction_name(), ins=[], outs=[],
                act_func_set_id=6))

            keep_s = consts.tile([P, gtot], _F32)
            gath_s = consts.tile([P, gtot], _F32)
            sumexp = consts.tile([P, gtot], _F32)
            dummy = consts.tile([P, C], _F16, tag="dummy")
            dummy_b = consts.tile([P, C], _F16, tag="dummy_b")
            dums = [dummy, dummy_b]
            dummy32 = consts.tile([P, gtot], _F32, tag="dummy32")
            part = consts.tile([P, 2], _F32)
            keepg_neg = consts.tile([P, 1], _F32)

            def emit_consts():
                nc.sync.dma_start(keep_s[:], keepf.ap())
                nc.sync.dma_start(gath_s[:], gathf.ap())
                nc.vector.tensor_reduce(
                    part[:, 1:2], keep_s[:], axis=mybir.AxisListType.X,
                    op=mybir.AluOpType.add)
                nc.vector.tensor_tensor(
                    dummy32[:], keep_s[:], gath_s[:], op=mybir.AluOpType.mult)
                nc.vector.tensor_scalar(
                    dummy32[:], dummy32[:], -1.0, 0.0,
                    op0=mybir.AluOpType.mult, op1=mybir.AluOpType.add,
                    accum_out=keepg_neg[:])

            def emit_a(own_lo, ng, col, act_accum=False, split=None):
                xt = x8p.tile([P, ng, C], _F8)
                nc.sync.dma_start(xt[:], x8r[:, own_lo:own_lo + ng, :])
                if act_accum:
                    for k in range(ng):
                        et = e16p.tile([P, 1, C], _F16, tag="etacc")
                        nc.scalar.activation(
                            et[:, 0, :], xt[:, k, :],
                            mybir.ActivationFunctionType.Exp,
                            accum_out=sumexp[:, col + k:col + k + 1])
                    return
                et = e16p.tile([P, ng, C], _F16)
                k0 = 0
                for sub in (split or [ng]):
                    nc.scalar.activation(
                        et[:, k0:k0 + sub, :], xt[:, k0:k0 + sub, :],
                        mybir.ActivationFunctionType.Exp)
                    for k in range(k0, k0 + sub):
                        j = col + k
                        nc.vector.tensor_scalar(
                            dums[j % 2][:], et[:, k, :], 1.0, 0.0,
                            op0=mybir.AluOpType.mult, op1=mybir.AluOpType.add,
                            accum_out=sumexp[:, j:j + 1])
                    k0 += sub

            def emit_d(own_lo, ng, col):
                xt = x16p.tile([P, ng, C], _F16)
                nc.sync.dma_start(xt[:], x16r[:, own_lo:own_lo + ng, :])
                bt = bitp.tile([P, ng, C], _I16)
                nc.vector.tensor_scalar(
                    bt[:], xt[:], A_SCH, B_SCH,
                    op0=mybir.AluOpType.mult, op1=mybir.AluOpType.add)
                btf = bt[:].bitcast(_F16)
                for k in range(ng):
                    j = col + k
                    nc.vector.tensor_scalar(
                        dums[j % 2][:], btf[:, k, :], 1.0, 0.0,
                        op0=mybir.AluOpType.mult, op1=mybir.AluOpType.add,
                        accum_out=sumexp[:, j:j + 1])

            # phased Ln + masked reduce; literal TTR seeds, folded by tiny adds
            ce_run = [keepg_neg[:]]

            def emit_phase(lo, hi):
                ncol = hi - lo
                logz = consts.tile([P, ncol], _F32, tag=f"logz{lo}")
                nc.scalar.activation(
                    logz[:], sumexp[:, lo:hi], mybir.ActivationFunctionType.Ln)
                ce_t = consts.tile([P, 1], _F32, tag=f"ce{lo}")
                nc.vector.tensor_tensor(
                    dummy32[:, 0:ncol], logz[:], keep_s[:, lo:hi],
                    op=mybir.AluOpType.mult)
                nc.vector.tensor_scalar(
                    dummy32[:, 0:ncol], dummy32[:, 0:ncol], 1.0, 0.0,
                    op0=mybir.AluOpType.mult, op1=mybir.AluOpType.add,
                    accum_out=ce_t[:])
                if hi == gtot:
                    dst = part[:, 0:1]
                else:
                    dst_t = consts.tile([P, 1], _F32, tag=f"cerun{lo}")
                    dst = dst_t[:]
                nc.vector.tensor_tensor(
                    dst, ce_run[0], ce_t[:], op=mybir.AluOpType.add)
                ce_run[0] = dst

            qt = max(8, gtot // 4 // 8 * 8)
            phase_targets = [qt, 2 * qt, 3 * qt, gtot - 16]
            phase_targets = sorted({t for t in phase_targets if 0 < t < gtot})
            phase_emit_after = {}
            for t in phase_targets:
                idx = next(i for i, cc in enumerate(ncols_done_after) if cc >= t)
                phase_emit_after[min(idx + PHASE_LAG, len(layout) - 1)] = t
            phase_lo = 0

            first_a = next(i for i, e in enumerate(layout) if e[0] == "a")
            for ei, (kind, own_lo, ng, col) in enumerate(layout):
                last = ei == len(layout) - 1
                if kind == "a":
                    split = None
                    if ng == 8 and ei == first_a:
                        split = [4, 4]
                    elif ng == 8 and last:
                        split = [2, 2, 2, 2]
                    emit_a(own_lo, ng, col,
                           act_accum=False,
                           split=split)
                else:
                    emit_d(own_lo, ng, col)
                if ei == 1:
                    emit_consts()
                if ei in phase_emit_after:
                    t = phase_emit_after[ei]
                    emit_phase(phase_lo, t)
                    phase_lo = t

            emit_phase(phase_lo, gtot)
            nc.sync.dma_start(out.ap(), part[:])

    nc.compile()
    return nc


def kernel(cls_score, label, min_classes):
    cls_score = np.ascontiguousarray(np.asarray(cls_score, dtype=np.float32))
    label = np.asarray(label).astype(np.int64)
    min_classes = np.asarray(min_classes)

    keep = ~np.isin(label, min_classes)                        # [N] bool
    kept = np.nonzero(keep)[0]
    if kept.size == 0:
        return np.array(0.0, dtype=np.float32)

    per_core = -(-kept.size // NCORES)                         # ceil
    gtot = max(32, -(-per_core // (P * 8)) * 8)                # groups, %8
    cap = P * gtot

    global _cached_nc
    nc = _cached.get(gtot)
    if nc is None:
        nc = _cached[gtot] = _build_nc(gtot)
    _cached_nc = nc

    rpp8, rpp16, _, _ = _geom(gtot)
    layout = _col_layout(gtot)
    perm = np.empty(gtot, dtype=np.int64)
    for kind, own_lo, ng, col in layout:
        src = own_lo + (0 if kind == "a" else rpp8)
        perm[col:col + ng] = np.arange(src, src + ng)

    in_maps = []
    for s in range(NCORES):
        idx = kept[s * per_core:(s + 1) * per_core]
        n_c = idx.size
        xs = np.zeros((cap, C), dtype=np.float32)
        xs[:n_c] = cls_score[idx]
        ls = np.zeros(cap, dtype=np.int64)
        ls[:n_c] = label[idx]
        ks = np.zeros(cap, dtype=np.float32)
        ks[:n_c] = 1.0
        # partition-major [P, gtot]
        xs = xs.reshape(P, gtot, C)
        ls = ls.reshape(P, gtot)
        ks = ks.reshape(P, gtot)
        x8 = np.ascontiguousarray(xs[:, :rpp8]).reshape(P * rpp8, C)
        x16 = np.ascontiguousarray(xs[:, rpp8:]).reshape(P * rpp16, C)
        x8q = x8.astype(ml_dtypes.float8_e4m3)
        x16q = x16.astype(np.float16)
        # byte-identical to an on-device gather of the uploaded arrays
        gath_own = np.empty((P, gtot), dtype=np.float32)
        gath_own[:, :rpp8] = x8q[np.arange(P * rpp8),
                                 ls[:, :rpp8].reshape(-1)].astype(
            np.float32).reshape(P, rpp8)
        gath_own[:, rpp8:] = x16q[np.arange(P * rpp16),
                                  ls[:, rpp8:].reshape(-1)].astype(
            np.float32).reshape(P, rpp16)
        in_maps.append({
            "x8": x8q,
            "x16": x16q,
            "gathf": np.ascontiguousarray(gath_own[:, perm]),
            "keepf": np.ascontiguousarray(ks[:, perm]),
        })

    results = run_bass_kernel_spmd(nc, in_maps, core_ids=list(range(NCORES)))
    partials = np.stack([r["out"] for r in results.results])  # [8, P, 2]
    ce_sum = float(partials[:, :, 0].astype(np.float64).sum())
    keep_sum = float(partials[:, :, 1].astype(np.float64).sum())
    return np.array(LOSS_WEIGHT * ce_sum / max(keep_sum, 1.0), dtype=np.float32)
